# revision 58
# baseline (speedup 1.0000x reference)
"""Trainium2 Bass kernel for causal GQA self-attention with RoPE + QK-RMSNorm.

Model (reference):
  B=2, S=2048, HID=2048, H=16 query heads, HKV=4 kv heads, D=128.
  q = x @ Wq.T, k = x @ Wk.T, v = x @ Wv.T
  q,k <- rmsnorm(rope(q,k))  (per-head, after rope)
  causal softmax(q k^T / sqrt(D)) @ v, then out @ Wo.T

Sharding: 8 cores, (batch 2) x (kv-group 4): core c handles batch c//4 and kv
head g=c%4 (query heads 4g..4g+3).

The out-projection contracts only over the core's local 512 attention
features (partial products), and a per-chunk ReduceScatter over the four
same-batch cores sums the partials while scattering tokens - so no PE work
ever waits on a collective, unlike an AllGather of activations which stalls
the gathered out-proj behind the collective each chunk. Each core ends up
with a [128-token, 2048] slice of every 512-token chunk; the host stitches
the full output. Only the last chunk's ReduceScatter is exposed.

Pipeline per 512-token query chunk: project 4 token tiles (causal attention
for chunk qc only needs K/V/Q from token tiles <= 4qc+3), attention, partial
out-proj (stationary operands are the chunk's feature-major attention tiles,
still in SBUF), ReduceScatter. Attention exploits causality inside the
chunk: for diagonal key tiles only queries >= the tile offset are scored
(sub-sliced moving operand) and all-zero PV sub-matmuls are skipped.

Host passes x pre-transposed+bf16 (xT [HID,S]) so projections use xT tiles as
the stationary operand directly - no on-chip x transposes; k|v projections run
as one matmul (concatenated weights). RoPE + RMSNorm run in bf16 with
per-tile batched ops (ssq computed pre-rope: rotation preserves norms, cos/sin
pre-repeated per head so the DVE fast path applies); k-side rope runs on the
gpsimd(Pool) engine. Softmax needs no max-subtraction: QK-RMSNorm bounds
|q.k|/sqrt(D) <= sqrt(D). The denominator comes from a ones-column in V.
"""

import os
from contextlib import ExitStack

import numpy as np
import ml_dtypes

# bass_utils unconditionally imports antenv.axon_hooks on the trace path;
# provide a no-op registry if the image's antenv lacks that module so a
# trace request degrades to "no profile" instead of crashing.
try:
    import antenv.axon_hooks  # noqa: F401
except ImportError:
    import sys as _sys
    import types as _types

    _m = _types.ModuleType("antenv.axon_hooks")
    _m._hook = None
    _m.set_axon_ntff_profile_hook = lambda h: setattr(_m, "_hook", h)
    _m.get_axon_ntff_profile_hook = lambda: getattr(_m, "_hook", None)
    _sys.modules["antenv.axon_hooks"] = _m

import concourse.bacc as bacc
import concourse.tile as tile
from concourse import bass_isa, mybir
from concourse.bass_utils import run_bass_kernel_spmd
from concourse.masks import make_identity

F32 = mybir.dt.float32
BF16 = mybir.dt.bfloat16

B, S, HID = 2, 2048, 2048
H, HKV, D = 16, 4, 128
G = HKV                 # kv groups == cores per batch
HL = H // HKV           # query heads per attention core
FQ = HL * D             # 512: local attention feature width
P = 128
NT = S // P             # 16 token tiles
NK = HID // P           # 16 contraction chunks
QCW = 512               # query-chunk width in the attention inner loop
NQC = S // QCW
NOB = HID // QCW        # 4: 512-wide out-proj column banks
SCALE = float(D) ** -0.5
EPS = float(np.finfo(np.float32).eps)

AluOp = mybir.AluOpType
Act = mybir.ActivationFunctionType
AxisX = mybir.AxisListType.X

# query pieces (start token, width); the trailing 256-halves shrink the
# exposed final ReduceScatter
PIECE_LIST = [(0, QCW), (QCW, QCW), (2 * QCW, QCW),
              (3 * QCW, QCW // 2), (3 * QCW + QCW // 2, QCW // 2)]


def _build_nc():
    phases = int(os.environ.get("KERNEL_PHASES", "4"))
    nc = bacc.Bacc("TRN2", target_bir_lowering=False, debug=False, num_devices=8)

    # x tile-major: row m*128+p (p = hid%128 within tile m), col c*128+t, so
    # one token tile is 128 partitions x 4KB contiguous — big DMA descriptors
    xT = nc.dram_tensor("xT", [NT * P, NK * P], BF16, kind="ExternalInput").ap()
    wqT = nc.dram_tensor("wqT", [HID, FQ], BF16, kind="ExternalInput").ap()
    wkvT = nc.dram_tensor("wkvT", [HID, 2 * D], BF16, kind="ExternalInput").ap()
    woT = nc.dram_tensor("woT", [FQ, HID], BF16, kind="ExternalInput").ap()
    cos = nc.dram_tensor("cos", [S, HL, D // 2], BF16, kind="ExternalInput").ap()
    sin = nc.dram_tensor("sin", [S, HL, D // 2], BF16, kind="ExternalInput").ap()
    masks = nc.dram_tensor("masks", [2, P, 2 * QCW], BF16, kind="ExternalInput").ap()
    out = nc.dram_tensor("out", [NQC * P, HID], BF16, kind="ExternalOutput").ap()

    with tile.TileContext(nc) as tc, ExitStack() as ctx:
        dram = ctx.enter_context(tc.tile_pool(name="dram", bufs=1, space="DRAM"))
        const = ctx.enter_context(tc.tile_pool(name="const", bufs=1))

        # attention/out-proj/ReduceScatter run per query piece. The last 512
        # tokens split into two 256-token halves: the first half's RS overlaps
        # the second half's compute, so only a 1MB (not 2MB) RS is exposed.
        # Each ncfw collective op has a ~12us latency floor, so the earlier
        # pieces stay at 512 tokens (fewer, larger ops).
        PIECES = PIECE_LIST
        part_ch = [dram.tile([qw, HID], BF16, name=f"part{i}")
                   for i, (q0, qw) in enumerate(PIECES)]
        # collectives read/write internal DRAM bounce tensors, not IO tensors
        rs_out = [dram.tile([qw // G, HID], BF16, name=f"rsout{i}")
                  for i, (q0, qw) in enumerate(PIECES)]

        # ---- constants ----------------------------------------------------
        # DMA issue order and queue choice matter at startup: wq/wkv gate the
        # first projection matmuls, so they go first on the sync queue; the
        # rest is needed later and rides the vector/scalar queues.
        wpool = ctx.enter_context(tc.tile_pool(name="wts", bufs=1))
        wq_sb = wpool.tile([P, NK, FQ], BF16, name="wq_sb")
        for c in range(NK):
            nc.sync.dma_start(
                out=wq_sb[:, c, :], in_=wqT[c * P:(c + 1) * P, :])
        wkv_sb = wpool.tile([P, NK, 2 * D], BF16, name="wkv_sb")
        nc.sync.dma_start(
            out=wkv_sb[:], in_=wkvT.rearrange("(c p) n -> p c n", p=P))

        # whole x resident in SBUF (no ring stalls); per-tile DMAs issue with
        # a 4-tile prefetch distance so the 8MB doesn't pile onto HBM during
        # the startup weight loads, yet stays ~20us ahead of consumption
        XPREF = 4
        xfull = wpool.tile([P, NT, NK, P], BF16, name="xfull")

        def load_x(m):
            nc.gpsimd.dma_start(
                out=xfull[:, m],
                in_=xT[m * P:(m + 1) * P, :].rearrange("p (c t) -> p c t", c=NK))

        for m in range(XPREF):
            load_x(m)



        ident = const.tile([P, P], BF16, name="ident")
        make_identity(nc, ident)
        epsb = const.tile([P, 1], F32, name="epsb")
        nc.vector.memset(epsb[:], EPS)

        cos_sb = const.tile([P, NT, HL, D // 2], BF16, name="cos_sb")
        nc.scalar.dma_start(
            out=cos_sb[:], in_=cos.rearrange("(m p) h d -> p m h d", p=P))
        sin_sb = const.tile([P, NT, HL, D // 2], BF16, name="sin_sb")
        nc.scalar.dma_start(
            out=sin_sb[:], in_=sin.rearrange("(m p) h d -> p m h d", p=P))
        mask_sb = const.tile([P, 2, 2 * QCW], BF16, name="mask_sb")
        nc.scalar.dma_start(out=mask_sb[:], in_=masks.rearrange("j p f -> p j f"))

        # wo on the scalar queue after cos/sin/mask: each HW DMA queue tops
        # out near ~100GB/s, so x must keep the gpsimd queue to itself
        wo_sb = const.tile([P, HL, HID], BF16, name="wo_sb")
        nc.scalar.dma_start(
            out=wo_sb[:], in_=woT.rearrange("(h p) n -> p h n", p=P))

        qTall = const.tile([P, HL, S], BF16, name="qTall")
        kT = const.tile([P, S], BF16, name="kT")
        vsb = [const.tile([P, D], BF16, name=f"vsb{m}") for m in range(NT)]
        ones_col = const.tile([P, 1], BF16, name="ones_col")
        nc.vector.memset(ones_col[:], 1.0)
        wkp = ctx.enter_context(tc.tile_pool(name="pwork", bufs=2))
        pq = ctx.enter_context(tc.tile_pool(name="pq", bufs=1, space="PSUM"))
        tps = ctx.enter_context(tc.tile_pool(name="tps", bufs=1, space="PSUM"))
        stp = ctx.enter_context(tc.tile_pool(name="stp", bufs=2, space="PSUM"))
        ovp = ctx.enter_context(tc.tile_pool(name="ovp", bufs=2, space="PSUM"))
        dpp = ctx.enter_context(tc.tile_pool(name="dpp", bufs=1, space="PSUM"))
        epool = ctx.enter_context(tc.tile_pool(name="epool", bufs=3))
        dsp = ctx.enter_context(tc.tile_pool(name="dsp", bufs=2))
        rbp = ctx.enter_context(tc.tile_pool(name="rbp", bufs=2))
        att = ctx.enter_context(tc.tile_pool(name="att", bufs=2 * HL))
        osb = ctx.enter_context(tc.tile_pool(name="osb", bufs=2))

        def proj_tile(m):
            # hid-major slice of x for this token tile, already resident:
            # [128 hid, NK chunks, 128 tokens]
            if m + XPREF < NT:
                load_x(m + XPREF)
            xt = xfull[:, m]

            q_ps = pq.tile([P, FQ], F32, tag="q", name=f"q_ps{m}")
            kv_ps = pq.tile([P, 2 * D], F32, tag="kv", name=f"kv_ps{m}")
            for c in range(NK):
                st_ = (c == 0)
                sp_ = (c == NK - 1)
                nc.tensor.matmul(q_ps[:], xt[:, c, :], wq_sb[:, c, :], start=st_, stop=sp_)
                nc.tensor.matmul(kv_ps[:], xt[:, c, :], wkv_sb[:, c, :], start=st_, stop=sp_)

            # casts PSUM->SBUF bf16 on the scalar engine (keeps DVE free)
            qsb = wkp.tile([P, FQ], BF16, tag="qsb", name=f"qsb{m}")
            nc.scalar.copy(out=qsb[:], in_=q_ps[:])
            ksb = wkp.tile([P, D], BF16, tag="ksb", name=f"ksb{m}")
            nc.scalar.copy(out=ksb[:], in_=kv_ps[:, 0:D])
            nc.scalar.copy(out=vsb[m][:], in_=kv_ps[:, D:2 * D])

            # sum-of-squares per head, computed pre-rope (rope is a rotation:
            # it preserves per-head norms)
            sq = wkp.tile([P, FQ], BF16, tag="sq", name=f"sq{m}")
            nc.vector.tensor_mul(out=sq[:], in0=qsb[:], in1=qsb[:])
            ss = wkp.tile([P, 8], F32, tag="ss", name=f"ss{m}")
            nc.vector.tensor_reduce(
                out=ss[:, 0:HL], in_=sq.rearrange("p (h d) -> p h d", h=HL),
                axis=AxisX, op=AluOp.add)
            sqk = wkp.tile([P, D], BF16, tag="sqk", name=f"sqk{m}")
            nc.vector.tensor_mul(out=sqk[:], in0=ksb[:], in1=ksb[:])
            nc.vector.tensor_reduce(
                out=ss[:, HL:HL + 1], in_=sqk[:], axis=AxisX, op=AluOp.add)
            rs = wkp.tile([P, 8], F32, tag="rs", name=f"rs{m}")
            nc.scalar.activation(
                out=rs[:, 0:HL + 1], in_=ss[:, 0:HL + 1], func=Act.Sqrt,
                scale=1.0 / D, bias=epsb[:])
            rr = wkp.tile([P, 8], F32, tag="rr", name=f"rr{m}")
            nc.vector.reciprocal(out=rr[:, 0:HL + 1], in_=rs[:, 0:HL + 1])

            # rope on q (4 heads at once, bf16, head-repeated cos/sin)
            cosb = cos_sb[:, m, :, :]
            sinb = sin_sb[:, m, :, :]
            qv = qsb.rearrange("p (h two d) -> p h two d", h=HL, two=2)
            qx1 = qv[:, :, 0, :]
            qx2 = qv[:, :, 1, :]
            qn = wkp.tile([P, FQ], BF16, tag="qn", name=f"qn{m}")
            qnv = qn.rearrange("p (h two d) -> p h two d", h=HL, two=2)
            t1 = wkp.tile([P, HL, D // 2], BF16, tag="t1", name=f"t1_{m}")
            t2 = wkp.tile([P, HL, D // 2], BF16, tag="t2", name=f"t2_{m}")
            nc.vector.tensor_mul(out=t1[:], in0=qx1, in1=cosb)
            nc.vector.tensor_mul(out=t2[:], in0=qx2, in1=sinb)
            nc.vector.tensor_add(out=qnv[:, :, 0, :], in0=t1[:], in1=t2[:])
            nc.vector.tensor_mul(out=t1[:], in0=qx2, in1=cosb)
            nc.vector.tensor_mul(out=t2[:], in0=qx1, in1=sinb)
            nc.vector.tensor_sub(out=qnv[:, :, 1, :], in0=t1[:], in1=t2[:])
            qb = wkp.tile([P, FQ], BF16, tag="qb", name=f"qb{m}")
            rrq = rr[:, 0:HL].unsqueeze(2).broadcast_to([P, HL, D])
            nc.vector.tensor_mul(
                out=qb.rearrange("p (h d) -> p h d", h=HL),
                in0=qn.rearrange("p (h d) -> p h d", h=HL), in1=rrq)

            # rope + normalize on k (DVE: gpsimd stays empty so the blocking
            # collective triggers there never stall compute)
            kv_ = ksb.rearrange("p (two d) -> p two d", two=2)
            cosk = cos_sb[:, m, 0, :]
            sink = sin_sb[:, m, 0, :]
            kn = wkp.tile([P, D], BF16, tag="kn", name=f"kn{m}")
            knv = kn.rearrange("p (two d) -> p two d", two=2)
            u1 = wkp.tile([P, D // 2], BF16, tag="u1", name=f"u1_{m}")
            u2 = wkp.tile([P, D // 2], BF16, tag="u2", name=f"u2_{m}")
            nc.vector.tensor_mul(out=u1[:], in0=kv_[:, 0, :], in1=cosk)
            nc.vector.tensor_mul(out=u2[:], in0=kv_[:, 1, :], in1=sink)
            nc.vector.tensor_add(out=knv[:, 0, :], in0=u1[:], in1=u2[:])
            nc.vector.tensor_mul(out=u1[:], in0=kv_[:, 1, :], in1=cosk)
            nc.vector.tensor_mul(out=u2[:], in0=kv_[:, 0, :], in1=sink)
            nc.vector.tensor_sub(out=knv[:, 1, :], in0=u1[:], in1=u2[:])
            kb = wkp.tile([P, D], BF16, tag="kb", name=f"kb{m}")
            rrk = rr[:, HL:HL + 1].broadcast_to([P, D])
            nc.vector.tensor_mul(out=kb[:], in0=kn[:], in1=rrk)

            # PE transposes into qTall / kT
            qtp = tps.tile([P, HL * P], BF16, tag="tp", name=f"qtp{m}")
            for h in range(HL):
                nc.tensor.transpose(
                    qtp[:, h * P:(h + 1) * P], qb[:, h * D:(h + 1) * D], ident[:])
            nc.scalar.copy(
                out=qTall[:, :, m * P:(m + 1) * P],
                in_=qtp.rearrange("p (h w) -> p h w", h=HL))
            tpk = tps.tile([P, P], BF16, tag="tp", name=f"tpk{m}")
            nc.tensor.transpose(tpk[:], kb[:], ident[:])
            nc.scalar.copy(out=kT[:, m * P:(m + 1) * P], in_=tpk[:])

        att_tiles = {}

        def attn_piece(pi, q0, qw):
            kb0 = q0 // P          # first diagonal key tile
            nkb = (q0 + qw) // P
            nqt = qw // P          # query sub-tiles in this piece
            for h in range(HL):
                # PV with V stationary: one wide matmul per key tile
                # accumulating feature-major oT [D, queries] in PSUM — no
                # per-query-subtile LDWEIGHTS, no output transposes. The
                # softmax denominator accumulates in parallel as an M=1
                # ones-column matmul chain in its own PSUM bank.
                ov = ovp.tile([P, QCW], F32, tag="ov", name=f"ov{pi}_{h}")
                dn = dpp.tile([P, QCW], F32, tag="dn", name=f"dn{pi}_{h}")
                for kb in range(nkb):
                    # diagonal key tile kb (offset j>=0 into the piece): only
                    # queries f >= j*128 can attend to it
                    j = kb - kb0
                    f0 = max(j, 0) * P
                    st = stp.tile([P, QCW], F32, tag="st", name=f"st{pi}_{h}_{kb}")
                    nc.tensor.matmul(
                        st[:, f0:qw], kT[:, kb * P:(kb + 1) * P],
                        qTall[:, h, q0 + f0:q0 + qw],
                        start=True, stop=True)
                    ex = epool.tile([P, QCW], BF16, tag="ex",
                                    name=f"ex{pi}_{h}_{kb}")
                    nc.scalar.activation(
                        out=ex[:, f0:qw], in_=st[:, f0:qw], func=Act.Exp,
                        scale=SCALE)
                    if j >= 0:
                        nc.vector.tensor_mul(
                            out=ex[:, f0:qw], in0=ex[:, f0:qw],
                            in1=mask_sb[:, j // 2,
                                        (j % 2) * QCW + f0:(j % 2) * QCW + qw])
                    nc.tensor.matmul(
                        ov[:, f0:qw], vsb[kb][:], ex[:, f0:qw],
                        start=(kb == 0), stop=(kb == nkb - 1))
                    nc.tensor.matmul(
                        dn[0:1, f0:qw], ones_col[:], ex[:, f0:qw],
                        start=(kb == 0), stop=(kb == nkb - 1))
                # 1/denominator on the single valid partition row, then
                # broadcast across partitions (gpsimd custom op) and scale
                rds = dsp.tile([1, QCW], F32, tag="ds", name=f"ds{pi}_{h}")
                nc.vector.reciprocal(out=rds[:, 0:qw], in_=dn[0:1, 0:qw])
                rdb = rbp.tile([P, QCW], F32, tag="rb", name=f"rb{pi}_{h}")
                nc.gpsimd.partition_broadcast(rdb[:, 0:qw], rds[:, 0:qw])
                att_h = att.tile([P, QCW], BF16, tag="attn", name=f"attn{pi}_{h}")
                nc.vector.tensor_mul(
                    out=att_h[:, 0:qw], in0=ov[:, 0:qw], in1=rdb[:, 0:qw])
                att_tiles[(pi, h)] = att_h

        grp = [[0, 1, 2, 3], [4, 5, 6, 7]]

        def outproj_piece(pi, q0, qw, ro, phases):
            # partial out-proj: contract only the local 512 features; the
            # ReduceScatter across the 4 same-batch cores supplies the rest
            # (and scatters qw/4 tokens back to each core).
            stores = []
            for m in range(qw // P):
                for ob in range(NOB):
                    # shares the scores PSUM ring: attention and out-proj are
                    # phase-disjoint, so the 2-deep ring pipelines across them
                    po = stp.tile([P, QCW], F32, tag="st", name=f"po{pi}_{m}_{ob}")
                    for h in range(HL):
                        nc.tensor.matmul(
                            po[:], att_tiles[(pi, h)][:, m * P:(m + 1) * P],
                            wo_sb[:, h, ob * QCW:(ob + 1) * QCW],
                            start=(h == 0), stop=(h == HL - 1))
                    ot = osb.tile([P, QCW], BF16, tag="ot",
                                  name=f"ot{pi}_{m}_{ob}")
                    nc.vector.tensor_copy(out=ot[:], in_=po[:])
                    stores.append(nc.sync.dma_start(
                        out=part_ch[pi][m * P:(m + 1) * P,
                                        ob * QCW:(ob + 1) * QCW],
                        in_=ot[:]))
            if phases >= 4:
                cc = nc.gpsimd.collective_compute(
                    "ReduceScatter", AluOp.add, replica_groups=grp,
                    ins=[part_ch[pi].opt()], outs=[rs_out[pi].opt()])
                for st_ in stores:
                    tile.add_dep_helper(
                        cc.ins, st_.ins, sync=True,
                        reason="ReduceScatter reads partial stores")
                # out copy rides the gpsimd stream: it must wait for the
                # collective anyway, and gpsimd (which blocks on cc) carries
                # no compute — on sync it head-of-line blocked the next
                # piece's partial stores for the whole collective duration
                dinst = nc.gpsimd.dma_start(
                    out=out[ro:ro + qw // G, :], in_=rs_out[pi][:])
                tile.add_dep_helper(
                    dinst.ins, cc.ins, sync=True,
                    reason="out copy reads ReduceScatter output")

        done = 0
        ro = 0
        for pi, (q0, qw) in enumerate(PIECES):
            for m in range(done, (q0 + qw) // P):
                proj_tile(m)
            done = (q0 + qw) // P
            if phases >= 2:
                attn_piece(pi, q0, qw)
                if phases >= 3:
                    outproj_piece(pi, q0, qw, ro, phases)
            ro += qw // G

    nc.compile()
    return nc


_NC_CACHE = {}


def _get_nc():
    if "nc" not in _NC_CACHE:
        _NC_CACHE["nc"] = _build_nc()
    return _NC_CACHE["nc"]


def _make_masks():
    # masks[jj2][p, jj*QCW + f] = 1 iff query f >= key offset (2*jj2+jj)*128+p
    out = np.zeros((2, P, 2 * QCW), dtype=np.float32)
    p = np.arange(P)[:, None]
    f = np.arange(QCW)[None, :]
    for jj2 in range(2):
        for jj in range(2):
            j = 2 * jj2 + jj
            out[jj2][:, jj * QCW:(jj + 1) * QCW] = (f >= j * P + p)
    return out.astype(ml_dtypes.bfloat16)


def kernel(**inputs):
    x = np.asarray(inputs["x"], np.float32)
    cos = np.asarray(inputs["cos"], np.float32).reshape(S, D // 2)
    sin = np.asarray(inputs["sin"], np.float32).reshape(S, D // 2)
    Wq = np.asarray(inputs["Wq"], np.float32)
    Wk = np.asarray(inputs["Wk"], np.float32)
    Wv = np.asarray(inputs["Wv"], np.float32)
    Wo = np.asarray(inputs["Wo"], np.float32)

    masks = _make_masks()
    bf = ml_dtypes.bfloat16

    # tile-major x: arr[m, p, c, t] = x[b][m*128+t, c*128+p] so a token tile's
    # DMA reads 4KB contiguous per partition
    xTb = [
        np.ascontiguousarray(
            x[b].reshape(NT, P, NK, P).transpose(0, 3, 2, 1)
        ).reshape(NT * P, NK * P).astype(bf)
        for b in range(B)
    ]
    cosb = np.ascontiguousarray(
        np.repeat(cos[:, None, :], HL, axis=1)).astype(bf)
    sinb = np.ascontiguousarray(
        np.repeat(sin[:, None, :], HL, axis=1)).astype(bf)

    in_maps = []
    for c in range(8):
        b, g = divmod(c, G)
        in_maps.append({
            "xT": xTb[b],
            "wqT": np.ascontiguousarray(Wq[g * FQ:(g + 1) * FQ, :].T).astype(bf),
            "wkvT": np.ascontiguousarray(np.concatenate([
                Wk[g * D:(g + 1) * D, :].T,
                Wv[g * D:(g + 1) * D, :].T], axis=1)).astype(bf),
            "woT": np.ascontiguousarray(Wo[:, g * FQ:(g + 1) * FQ].T).astype(bf),
            "cos": cosb,
            "sin": sinb,
            "masks": masks,
        })

    nc = _get_nc()
    trace = bool(int(os.environ.get("KERNEL_TRACE", "0")))
    tmpdir = os.environ.get("KERNEL_TMPDIR") or None
    res = run_bass_kernel_spmd(
        nc, in_maps, core_ids=list(range(8)), trace=trace, tmpdir=tmpdir)
    kernel.exec_time_ns = res.exec_time_ns
    kernel.last_result = res

    out = np.empty((B, S, HID), np.float32)
    for c in range(8):
        b, r = divmod(c, G)
        o = np.asarray(res.results[c]["out"]).astype(np.float32)
        ro = 0
        for q0, qw in PIECE_LIST:
            n = qw // G
            t0 = q0 + r * n
            out[b, t0:t0 + n, :] = o[ro:ro + n]
            ro += n
    return out


# revision 59
# speedup vs baseline: 1.1638x; 1.1638x over previous
"""Trainium2 Bass kernel for causal GQA self-attention with RoPE + QK-RMSNorm.

Model (reference):
  B=2, S=2048, HID=2048, H=16 query heads, HKV=4 kv heads, D=128.
  q = x @ Wq.T, k = x @ Wk.T, v = x @ Wv.T
  q,k <- rmsnorm(rope(q,k))  (per-head, after rope)
  causal softmax(q k^T / sqrt(D)) @ v, then out @ Wo.T

Sharding: 8 cores, (batch 2) x (kv-group 4): core c handles batch c//4 and kv
head g=c%4 (query heads 4g..4g+3).

The out-projection contracts only over the core's local 512 attention
features (partial products), and a per-chunk ReduceScatter over the four
same-batch cores sums the partials while scattering tokens - so no PE work
ever waits on a collective, unlike an AllGather of activations which stalls
the gathered out-proj behind the collective each chunk. Each core ends up
with a [128-token, 2048] slice of every 512-token chunk; the host stitches
the full output. Only the last chunk's ReduceScatter is exposed.

Pipeline per 512-token query chunk: project 4 token tiles (causal attention
for chunk qc only needs K/V/Q from token tiles <= 4qc+3), attention, partial
out-proj (stationary operands are the chunk's feature-major attention tiles,
still in SBUF), ReduceScatter. Attention exploits causality inside the
chunk: for diagonal key tiles only queries >= the tile offset are scored
(sub-sliced moving operand) and all-zero PV sub-matmuls are skipped.

Host passes x pre-transposed+bf16 (xT [HID,S]) so projections use xT tiles as
the stationary operand directly - no on-chip x transposes; k|v projections run
as one matmul (concatenated weights). RoPE + RMSNorm run in bf16 with
per-tile batched ops (ssq computed pre-rope: rotation preserves norms, cos/sin
pre-repeated per head so the DVE fast path applies); k-side rope runs on the
gpsimd(Pool) engine. Softmax needs no max-subtraction: QK-RMSNorm bounds
|q.k|/sqrt(D) <= sqrt(D). The denominator comes from a ones-column in V.
"""

import os
from contextlib import ExitStack

import numpy as np
import ml_dtypes

# bass_utils unconditionally imports antenv.axon_hooks on the trace path;
# provide a no-op registry if the image's antenv lacks that module so a
# trace request degrades to "no profile" instead of crashing.
try:
    import antenv.axon_hooks  # noqa: F401
except ImportError:
    import sys as _sys
    import types as _types

    _m = _types.ModuleType("antenv.axon_hooks")
    _m._hook = None
    _m.set_axon_ntff_profile_hook = lambda h: setattr(_m, "_hook", h)
    _m.get_axon_ntff_profile_hook = lambda: getattr(_m, "_hook", None)
    _sys.modules["antenv.axon_hooks"] = _m

import concourse.bacc as bacc
import concourse.tile as tile
from concourse import bass_isa, mybir
from concourse.bass_utils import run_bass_kernel_spmd
from concourse.masks import make_identity

F32 = mybir.dt.float32
BF16 = mybir.dt.bfloat16

B, S, HID = 2, 2048, 2048
H, HKV, D = 16, 4, 128
G = HKV                 # kv groups == cores per batch
HL = H // HKV           # query heads per attention core
FQ = HL * D             # 512: local attention feature width
P = 128
NT = S // P             # 16 token tiles
NK = HID // P           # 16 contraction chunks
QCW = 512               # query-chunk width in the attention inner loop
NQC = S // QCW
NOB = HID // QCW        # 4: 512-wide out-proj column banks
SCALE = float(D) ** -0.5
EPS = float(np.finfo(np.float32).eps)

AluOp = mybir.AluOpType
Act = mybir.ActivationFunctionType
AxisX = mybir.AxisListType.X

# query pieces (start token, width); the trailing 256-halves shrink the
# exposed final ReduceScatter
PIECE_LIST = [(0, QCW), (QCW, QCW), (2 * QCW, QCW),
              (3 * QCW, QCW // 2), (3 * QCW + QCW // 2, QCW // 2)]


def _build_nc():
    phases = int(os.environ.get("KERNEL_PHASES", "4"))
    nc = bacc.Bacc("TRN2", target_bir_lowering=False, debug=False, num_devices=8)

    # x tile-major: row m*128+p (p = hid%128 within tile m), col c*128+t, so
    # one token tile is 128 partitions x 4KB contiguous — big DMA descriptors
    xT = nc.dram_tensor("xT", [NT * P, NK * P], BF16, kind="ExternalInput").ap()
    wqT = nc.dram_tensor("wqT", [HID, FQ], BF16, kind="ExternalInput").ap()
    wkvT = nc.dram_tensor("wkvT", [HID, 2 * D], BF16, kind="ExternalInput").ap()
    woT = nc.dram_tensor("woT", [FQ, HID], BF16, kind="ExternalInput").ap()
    cos = nc.dram_tensor("cos", [S, HL, D // 2], BF16, kind="ExternalInput").ap()
    sin = nc.dram_tensor("sin", [S, HL, D // 2], BF16, kind="ExternalInput").ap()
    masks = nc.dram_tensor("masks", [2, P, 2 * QCW], BF16, kind="ExternalInput").ap()
    out = nc.dram_tensor("out", [NQC * P, HID], BF16, kind="ExternalOutput").ap()

    with tile.TileContext(nc) as tc, ExitStack() as ctx:
        dram = ctx.enter_context(tc.tile_pool(name="dram", bufs=1, space="DRAM"))
        const = ctx.enter_context(tc.tile_pool(name="const", bufs=1))

        # attention/out-proj/ReduceScatter run per query piece. The last 512
        # tokens split into two 256-token halves: the first half's RS overlaps
        # the second half's compute, so only a 1MB (not 2MB) RS is exposed.
        # Each ncfw collective op has a ~12us latency floor, so the earlier
        # pieces stay at 512 tokens (fewer, larger ops).
        PIECES = PIECE_LIST
        part_ch = [dram.tile([qw, HID], BF16, name=f"part{i}")
                   for i, (q0, qw) in enumerate(PIECES)]
        # collectives read/write internal DRAM bounce tensors, not IO tensors
        rs_out = [dram.tile([qw // G, HID], BF16, name=f"rsout{i}")
                  for i, (q0, qw) in enumerate(PIECES)]

        # ---- constants ----------------------------------------------------
        # DMA issue order and queue choice matter at startup: wq/wkv gate the
        # first projection matmuls, so they go first on the sync queue; the
        # rest is needed later and rides the vector/scalar queues.
        wpool = ctx.enter_context(tc.tile_pool(name="wts", bufs=1))
        wq_sb = wpool.tile([P, NK, FQ], BF16, name="wq_sb")
        for c in range(NK):
            nc.sync.dma_start(
                out=wq_sb[:, c, :], in_=wqT[c * P:(c + 1) * P, :])
        wkv_sb = wpool.tile([P, NK, 2 * D], BF16, name="wkv_sb")
        nc.sync.dma_start(
            out=wkv_sb[:], in_=wkvT.rearrange("(c p) n -> p c n", p=P))

        # whole x resident in SBUF (no ring stalls); per-tile DMAs issue with
        # a 4-tile prefetch distance so the 8MB doesn't pile onto HBM during
        # the startup weight loads, yet stays ~20us ahead of consumption
        XPREF = 4
        xfull = wpool.tile([P, NT, NK, P], BF16, name="xfull")

        def load_x(m):
            nc.gpsimd.dma_start(
                out=xfull[:, m],
                in_=xT[m * P:(m + 1) * P, :].rearrange("p (c t) -> p c t", c=NK))

        for m in range(XPREF):
            load_x(m)



        ident = const.tile([P, P], BF16, name="ident")
        make_identity(nc, ident)
        epsb = const.tile([P, 1], F32, name="epsb")
        nc.vector.memset(epsb[:], EPS)

        cos_sb = const.tile([P, NT, HL, D // 2], BF16, name="cos_sb")
        nc.scalar.dma_start(
            out=cos_sb[:], in_=cos.rearrange("(m p) h d -> p m h d", p=P))
        sin_sb = const.tile([P, NT, HL, D // 2], BF16, name="sin_sb")
        nc.scalar.dma_start(
            out=sin_sb[:], in_=sin.rearrange("(m p) h d -> p m h d", p=P))
        mask_sb = const.tile([P, 2, 2 * QCW], BF16, name="mask_sb")
        nc.scalar.dma_start(out=mask_sb[:], in_=masks.rearrange("j p f -> p j f"))

        # wo on the scalar queue after cos/sin/mask: each HW DMA queue tops
        # out near ~100GB/s, so x must keep the gpsimd queue to itself
        wo_sb = const.tile([P, HL, HID], BF16, name="wo_sb")
        nc.scalar.dma_start(
            out=wo_sb[:], in_=woT.rearrange("(h p) n -> p h n", p=P))

        qTall = const.tile([P, HL, S], BF16, name="qTall")
        kT = const.tile([P, S], BF16, name="kT")
        vext = [const.tile([P, 129], BF16, name=f"vext{m}") for m in range(NT)]
        for m in range(NT):
            nc.vector.memset(vext[m][:, D:D + 1], 1.0)
        wkp = ctx.enter_context(tc.tile_pool(name="pwork", bufs=2))
        pq = ctx.enter_context(tc.tile_pool(name="pq", bufs=1, space="PSUM"))
        tps = ctx.enter_context(tc.tile_pool(name="tps", bufs=1, space="PSUM"))
        stp = ctx.enter_context(tc.tile_pool(name="stp", bufs=2, space="PSUM"))
        opp = ctx.enter_context(tc.tile_pool(name="opp", bufs=2, space="PSUM"))
        epool = ctx.enter_context(tc.tile_pool(name="epool", bufs=3))
        asb = ctx.enter_context(tc.tile_pool(name="asb", bufs=4))
        rpool = ctx.enter_context(tc.tile_pool(name="rpool", bufs=4))
        att = ctx.enter_context(tc.tile_pool(name="att", bufs=2 * HL))
        osb = ctx.enter_context(tc.tile_pool(name="osb", bufs=2))

        def proj_tile(m):
            # hid-major slice of x for this token tile, already resident:
            # [128 hid, NK chunks, 128 tokens]
            if m + XPREF < NT:
                load_x(m + XPREF)
            xt = xfull[:, m]

            q_ps = pq.tile([P, FQ], F32, tag="q", name=f"q_ps{m}")
            kv_ps = pq.tile([P, 2 * D], F32, tag="kv", name=f"kv_ps{m}")
            for c in range(NK):
                st_ = (c == 0)
                sp_ = (c == NK - 1)
                nc.tensor.matmul(q_ps[:], xt[:, c, :], wq_sb[:, c, :], start=st_, stop=sp_)
                nc.tensor.matmul(kv_ps[:], xt[:, c, :], wkv_sb[:, c, :], start=st_, stop=sp_)

            # casts PSUM->SBUF bf16 on the scalar engine (keeps DVE free)
            qsb = wkp.tile([P, FQ], BF16, tag="qsb", name=f"qsb{m}")
            nc.scalar.copy(out=qsb[:], in_=q_ps[:])
            ksb = wkp.tile([P, D], BF16, tag="ksb", name=f"ksb{m}")
            nc.scalar.copy(out=ksb[:], in_=kv_ps[:, 0:D])
            nc.scalar.copy(out=vext[m][:, 0:D], in_=kv_ps[:, D:2 * D])

            # sum-of-squares per head, computed pre-rope (rope is a rotation:
            # it preserves per-head norms)
            sq = wkp.tile([P, FQ], BF16, tag="sq", name=f"sq{m}")
            nc.vector.tensor_mul(out=sq[:], in0=qsb[:], in1=qsb[:])
            ss = wkp.tile([P, 8], F32, tag="ss", name=f"ss{m}")
            nc.vector.tensor_reduce(
                out=ss[:, 0:HL], in_=sq.rearrange("p (h d) -> p h d", h=HL),
                axis=AxisX, op=AluOp.add)
            sqk = wkp.tile([P, D], BF16, tag="sqk", name=f"sqk{m}")
            nc.vector.tensor_mul(out=sqk[:], in0=ksb[:], in1=ksb[:])
            nc.vector.tensor_reduce(
                out=ss[:, HL:HL + 1], in_=sqk[:], axis=AxisX, op=AluOp.add)
            rs = wkp.tile([P, 8], F32, tag="rs", name=f"rs{m}")
            nc.scalar.activation(
                out=rs[:, 0:HL + 1], in_=ss[:, 0:HL + 1], func=Act.Sqrt,
                scale=1.0 / D, bias=epsb[:])
            rr = wkp.tile([P, 8], F32, tag="rr", name=f"rr{m}")
            nc.vector.reciprocal(out=rr[:, 0:HL + 1], in_=rs[:, 0:HL + 1])

            # rope on q (4 heads at once, bf16, head-repeated cos/sin)
            cosb = cos_sb[:, m, :, :]
            sinb = sin_sb[:, m, :, :]
            qv = qsb.rearrange("p (h two d) -> p h two d", h=HL, two=2)
            qx1 = qv[:, :, 0, :]
            qx2 = qv[:, :, 1, :]
            qn = wkp.tile([P, FQ], BF16, tag="qn", name=f"qn{m}")
            qnv = qn.rearrange("p (h two d) -> p h two d", h=HL, two=2)
            t1 = wkp.tile([P, HL, D // 2], BF16, tag="t1", name=f"t1_{m}")
            t2 = wkp.tile([P, HL, D // 2], BF16, tag="t2", name=f"t2_{m}")
            nc.vector.tensor_mul(out=t1[:], in0=qx1, in1=cosb)
            nc.vector.tensor_mul(out=t2[:], in0=qx2, in1=sinb)
            nc.vector.tensor_add(out=qnv[:, :, 0, :], in0=t1[:], in1=t2[:])
            nc.vector.tensor_mul(out=t1[:], in0=qx2, in1=cosb)
            nc.vector.tensor_mul(out=t2[:], in0=qx1, in1=sinb)
            nc.vector.tensor_sub(out=qnv[:, :, 1, :], in0=t1[:], in1=t2[:])
            qb = wkp.tile([P, FQ], BF16, tag="qb", name=f"qb{m}")
            rrq = rr[:, 0:HL].unsqueeze(2).broadcast_to([P, HL, D])
            nc.vector.tensor_mul(
                out=qb.rearrange("p (h d) -> p h d", h=HL),
                in0=qn.rearrange("p (h d) -> p h d", h=HL), in1=rrq)

            # rope + normalize on k (DVE: gpsimd stays empty so the blocking
            # collective triggers there never stall compute)
            kv_ = ksb.rearrange("p (two d) -> p two d", two=2)
            cosk = cos_sb[:, m, 0, :]
            sink = sin_sb[:, m, 0, :]
            kn = wkp.tile([P, D], BF16, tag="kn", name=f"kn{m}")
            knv = kn.rearrange("p (two d) -> p two d", two=2)
            u1 = wkp.tile([P, D // 2], BF16, tag="u1", name=f"u1_{m}")
            u2 = wkp.tile([P, D // 2], BF16, tag="u2", name=f"u2_{m}")
            nc.vector.tensor_mul(out=u1[:], in0=kv_[:, 0, :], in1=cosk)
            nc.vector.tensor_mul(out=u2[:], in0=kv_[:, 1, :], in1=sink)
            nc.vector.tensor_add(out=knv[:, 0, :], in0=u1[:], in1=u2[:])
            nc.vector.tensor_mul(out=u1[:], in0=kv_[:, 1, :], in1=cosk)
            nc.vector.tensor_mul(out=u2[:], in0=kv_[:, 0, :], in1=sink)
            nc.vector.tensor_sub(out=knv[:, 1, :], in0=u1[:], in1=u2[:])
            kb = wkp.tile([P, D], BF16, tag="kb", name=f"kb{m}")
            rrk = rr[:, HL:HL + 1].broadcast_to([P, D])
            nc.vector.tensor_mul(out=kb[:], in0=kn[:], in1=rrk)

            # PE transposes into qTall / kT
            qtp = tps.tile([P, HL * P], BF16, tag="tp", name=f"qtp{m}")
            for h in range(HL):
                nc.tensor.transpose(
                    qtp[:, h * P:(h + 1) * P], qb[:, h * D:(h + 1) * D], ident[:])
            nc.scalar.copy(
                out=qTall[:, :, m * P:(m + 1) * P],
                in_=qtp.rearrange("p (h w) -> p h w", h=HL))
            tpk = tps.tile([P, P], BF16, tag="tp", name=f"tpk{m}")
            nc.tensor.transpose(tpk[:], kb[:], ident[:])
            nc.scalar.copy(out=kT[:, m * P:(m + 1) * P], in_=tpk[:])

        att_tiles = {}

        def attn_piece(pi, q0, qw):
            kb0 = q0 // P          # first diagonal key tile
            nkb = (q0 + qw) // P
            nqt = qw // P          # query sub-tiles in this piece
            for h in range(HL):
                # 136-stride keeps the second accumulation region 16B-aligned
                osum = opp.tile([P, 2, 136], F32, tag="O", name=f"O{pi}_{h}_a")
                osum2 = opp.tile([P, 2, 136], F32, tag="O", name=f"O{pi}_{h}_b")
                otile = (osum, osum, osum2, osum2)
                for kb in range(nkb):
                    # diagonal key tile kb (offset j>=0 into the piece): only
                    # queries f >= j*128 can attend to it
                    j = kb - kb0
                    f0 = max(j, 0) * P
                    st = stp.tile([P, QCW], F32, tag="st", name=f"st{pi}_{h}_{kb}")
                    nc.tensor.matmul(
                        st[:, f0:qw], kT[:, kb * P:(kb + 1) * P],
                        qTall[:, h, q0 + f0:q0 + qw],
                        start=True, stop=True)
                    ex = epool.tile([P, QCW], BF16, tag="ex",
                                    name=f"ex{pi}_{h}_{kb}")
                    nc.scalar.activation(
                        out=ex[:, f0:qw], in_=st[:, f0:qw], func=Act.Exp,
                        scale=SCALE)
                    if j >= 0:
                        nc.vector.tensor_mul(
                            out=ex[:, f0:qw], in0=ex[:, f0:qw],
                            in1=mask_sb[:, j // 2,
                                        (j % 2) * QCW + f0:(j % 2) * QCW + qw])
                    for s in range(nqt):
                        if s < j:
                            continue  # query sub-tile fully before key tile
                        # start=True clears the whole PSUM bank's accumulation
                        # state: only the first chain on each bank (s even)
                        # may open the group; the sibling chain's first write
                        # lands in overwrite mode on the freshly cleared bank.
                        # Chain s's last contribution is its diagonal tile.
                        nc.tensor.matmul(
                            otile[s][:, s % 2, 0:129],
                            ex[:, s * P:(s + 1) * P],
                            vext[kb][:],
                            start=(kb == 0 and s % 2 == 0),
                            stop=(kb == kb0 + s))
                # normalize (per-query 1/denom), transpose to feature-major
                nA = min(nqt, 2)
                rcA = rpool.tile([P, 2], F32, tag="rcA", name=f"rcA{pi}_{h}")
                nc.vector.reciprocal(out=rcA[:, 0:nA], in_=osum[:, 0:nA, D])
                obA = asb.tile([P, 2, D], BF16, tag="obA", name=f"obA{pi}_{h}")
                nc.vector.tensor_mul(
                    out=obA[:, 0:nA], in0=osum[:, 0:nA, 0:D],
                    in1=rcA[:, 0:nA].unsqueeze(2).broadcast_to([P, nA, D]))
                obs = [obA[:, 0, :], obA[:, 1, :]]
                if nqt > 2:
                    rcB = rpool.tile([P, 2], F32, tag="rcB", name=f"rcB{pi}_{h}")
                    nc.vector.reciprocal(out=rcB[:], in_=osum2[:, :, D])
                    obB = asb.tile([P, 2, D], BF16, tag="obB",
                                   name=f"obB{pi}_{h}")
                    nc.vector.tensor_mul(
                        out=obB[:], in0=osum2[:, :, 0:D],
                        in1=rcB.unsqueeze(2).broadcast_to([P, 2, D]))
                    obs += [obB[:, 0, :], obB[:, 1, :]]
                to4 = tps.tile([P, QCW], BF16, tag="tp", name=f"to{pi}_{h}")
                for s in range(nqt):
                    nc.tensor.transpose(to4[:, s * P:(s + 1) * P], obs[s], ident[:])
                att_h = att.tile([P, QCW], BF16, tag="attn", name=f"attn{pi}_{h}")
                nc.vector.tensor_copy(out=att_h[:, 0:qw], in_=to4[:, 0:qw])
                att_tiles[(pi, h)] = att_h

        grp = [[0, 1, 2, 3], [4, 5, 6, 7]]

        def outproj_piece(pi, q0, qw, ro, phases):
            # partial out-proj: contract only the local 512 features; the
            # ReduceScatter across the 4 same-batch cores supplies the rest
            # (and scatters qw/4 tokens back to each core).
            stores = []
            for m in range(qw // P):
                for ob in range(NOB):
                    # shares the scores PSUM ring: attention and out-proj are
                    # phase-disjoint, so the 2-deep ring pipelines across them
                    po = stp.tile([P, QCW], F32, tag="st", name=f"po{pi}_{m}_{ob}")
                    for h in range(HL):
                        nc.tensor.matmul(
                            po[:], att_tiles[(pi, h)][:, m * P:(m + 1) * P],
                            wo_sb[:, h, ob * QCW:(ob + 1) * QCW],
                            start=(h == 0), stop=(h == HL - 1))
                    ot = osb.tile([P, QCW], BF16, tag="ot",
                                  name=f"ot{pi}_{m}_{ob}")
                    nc.vector.tensor_copy(out=ot[:], in_=po[:])
                    stores.append(nc.sync.dma_start(
                        out=part_ch[pi][m * P:(m + 1) * P,
                                        ob * QCW:(ob + 1) * QCW],
                        in_=ot[:]))
            if phases >= 4:
                cc = nc.gpsimd.collective_compute(
                    "ReduceScatter", AluOp.add, replica_groups=grp,
                    ins=[part_ch[pi].opt()], outs=[rs_out[pi].opt()])
                for st_ in stores:
                    tile.add_dep_helper(
                        cc.ins, st_.ins, sync=True,
                        reason="ReduceScatter reads partial stores")
                # out copy rides the gpsimd stream: it must wait for the
                # collective anyway, and gpsimd (which blocks on cc) carries
                # no compute — on sync it head-of-line blocked the next
                # piece's partial stores for the whole collective duration
                dinst = nc.gpsimd.dma_start(
                    out=out[ro:ro + qw // G, :], in_=rs_out[pi][:])
                tile.add_dep_helper(
                    dinst.ins, cc.ins, sync=True,
                    reason="out copy reads ReduceScatter output")

        done = 0
        ro = 0
        for pi, (q0, qw) in enumerate(PIECES):
            for m in range(done, (q0 + qw) // P):
                proj_tile(m)
            done = (q0 + qw) // P
            if phases >= 2:
                attn_piece(pi, q0, qw)
                if phases >= 3:
                    outproj_piece(pi, q0, qw, ro, phases)
            ro += qw // G

    nc.compile()
    return nc


_NC_CACHE = {}


def _get_nc():
    if "nc" not in _NC_CACHE:
        _NC_CACHE["nc"] = _build_nc()
    return _NC_CACHE["nc"]


def _make_masks():
    # masks[jj2][p, jj*QCW + f] = 1 iff query f >= key offset (2*jj2+jj)*128+p
    out = np.zeros((2, P, 2 * QCW), dtype=np.float32)
    p = np.arange(P)[:, None]
    f = np.arange(QCW)[None, :]
    for jj2 in range(2):
        for jj in range(2):
            j = 2 * jj2 + jj
            out[jj2][:, jj * QCW:(jj + 1) * QCW] = (f >= j * P + p)
    return out.astype(ml_dtypes.bfloat16)


def kernel(**inputs):
    x = np.asarray(inputs["x"], np.float32)
    cos = np.asarray(inputs["cos"], np.float32).reshape(S, D // 2)
    sin = np.asarray(inputs["sin"], np.float32).reshape(S, D // 2)
    Wq = np.asarray(inputs["Wq"], np.float32)
    Wk = np.asarray(inputs["Wk"], np.float32)
    Wv = np.asarray(inputs["Wv"], np.float32)
    Wo = np.asarray(inputs["Wo"], np.float32)

    masks = _make_masks()
    bf = ml_dtypes.bfloat16

    # tile-major x: arr[m, p, c, t] = x[b][m*128+t, c*128+p] so a token tile's
    # DMA reads 4KB contiguous per partition
    xTb = [
        np.ascontiguousarray(
            x[b].reshape(NT, P, NK, P).transpose(0, 3, 2, 1)
        ).reshape(NT * P, NK * P).astype(bf)
        for b in range(B)
    ]
    cosb = np.ascontiguousarray(
        np.repeat(cos[:, None, :], HL, axis=1)).astype(bf)
    sinb = np.ascontiguousarray(
        np.repeat(sin[:, None, :], HL, axis=1)).astype(bf)

    in_maps = []
    for c in range(8):
        b, g = divmod(c, G)
        in_maps.append({
            "xT": xTb[b],
            "wqT": np.ascontiguousarray(Wq[g * FQ:(g + 1) * FQ, :].T).astype(bf),
            "wkvT": np.ascontiguousarray(np.concatenate([
                Wk[g * D:(g + 1) * D, :].T,
                Wv[g * D:(g + 1) * D, :].T], axis=1)).astype(bf),
            "woT": np.ascontiguousarray(Wo[:, g * FQ:(g + 1) * FQ].T).astype(bf),
            "cos": cosb,
            "sin": sinb,
            "masks": masks,
        })

    nc = _get_nc()
    trace = bool(int(os.environ.get("KERNEL_TRACE", "0")))
    tmpdir = os.environ.get("KERNEL_TMPDIR") or None
    res = run_bass_kernel_spmd(
        nc, in_maps, core_ids=list(range(8)), trace=trace, tmpdir=tmpdir)
    kernel.exec_time_ns = res.exec_time_ns
    kernel.last_result = res

    out = np.empty((B, S, HID), np.float32)
    for c in range(8):
        b, r = divmod(c, G)
        o = np.asarray(res.results[c]["out"]).astype(np.float32)
        ro = 0
        for q0, qw in PIECE_LIST:
            n = qw // G
            t0 = q0 + r * n
            out[b, t0:t0 + n, :] = o[ro:ro + n]
            ro += n
    return out


# revision 61
# speedup vs baseline: 1.2632x; 1.0854x over previous
"""Trainium2 Bass kernel for causal GQA self-attention with RoPE + QK-RMSNorm.

Model (reference):
  B=2, S=2048, HID=2048, H=16 query heads, HKV=4 kv heads, D=128.
  q = x @ Wq.T, k = x @ Wk.T, v = x @ Wv.T
  q,k <- rmsnorm(rope(q,k))  (per-head, after rope)
  causal softmax(q k^T / sqrt(D)) @ v, then out @ Wo.T

Sharding: 8 cores, (batch 2) x (kv-group 4): core c handles batch c//4 and kv
head g=c%4 (query heads 4g..4g+3).

The out-projection contracts only over the core's local 512 attention
features (partial products), and a per-chunk ReduceScatter over the four
same-batch cores sums the partials while scattering tokens - so no PE work
ever waits on a collective, unlike an AllGather of activations which stalls
the gathered out-proj behind the collective each chunk. Each core ends up
with a [128-token, 2048] slice of every 512-token chunk; the host stitches
the full output. Only the last chunk's ReduceScatter is exposed.

Pipeline per 512-token query chunk: project 4 token tiles (causal attention
for chunk qc only needs K/V/Q from token tiles <= 4qc+3), attention, partial
out-proj (stationary operands are the chunk's feature-major attention tiles,
still in SBUF), ReduceScatter. Attention exploits causality inside the
chunk: for diagonal key tiles only queries >= the tile offset are scored
(sub-sliced moving operand) and all-zero PV sub-matmuls are skipped.

Host passes x pre-transposed+bf16 (xT [HID,S]) so projections use xT tiles as
the stationary operand directly - no on-chip x transposes; k|v projections run
as one matmul (concatenated weights). RoPE + RMSNorm run in bf16 with
per-tile batched ops (ssq computed pre-rope: rotation preserves norms, cos/sin
pre-repeated per head so the DVE fast path applies); k-side rope runs on the
gpsimd(Pool) engine. Softmax needs no max-subtraction: QK-RMSNorm bounds
|q.k|/sqrt(D) <= sqrt(D). The denominator comes from a ones-column in V.
"""

import os
from contextlib import ExitStack

import numpy as np
import ml_dtypes

# bass_utils unconditionally imports antenv.axon_hooks on the trace path;
# provide a no-op registry if the image's antenv lacks that module so a
# trace request degrades to "no profile" instead of crashing.
try:
    import antenv.axon_hooks  # noqa: F401
except ImportError:
    import sys as _sys
    import types as _types

    _m = _types.ModuleType("antenv.axon_hooks")
    _m._hook = None
    _m.set_axon_ntff_profile_hook = lambda h: setattr(_m, "_hook", h)
    _m.get_axon_ntff_profile_hook = lambda: getattr(_m, "_hook", None)
    _sys.modules["antenv.axon_hooks"] = _m

import concourse.bacc as bacc
import concourse.tile as tile
from concourse import bass_isa, mybir
from concourse.bass_utils import run_bass_kernel_spmd
from concourse.masks import make_identity

F32 = mybir.dt.float32
BF16 = mybir.dt.bfloat16

B, S, HID = 2, 2048, 2048
H, HKV, D = 16, 4, 128
G = HKV                 # kv groups == cores per batch
HL = H // HKV           # query heads per attention core
FQ = HL * D             # 512: local attention feature width
P = 128
NT = S // P             # 16 token tiles
NK = HID // P           # 16 contraction chunks
QCW = 512               # query-chunk width in the attention inner loop
NQC = S // QCW
NOB = HID // QCW        # 4: 512-wide out-proj column banks
SCALE = float(D) ** -0.5
EPS = float(np.finfo(np.float32).eps)

AluOp = mybir.AluOpType
Act = mybir.ActivationFunctionType
AxisX = mybir.AxisListType.X

# query pieces (start token, width); the trailing 256-halves shrink the
# exposed final ReduceScatter
PIECE_LIST = [(0, QCW), (QCW, QCW), (2 * QCW, QCW),
              (3 * QCW, QCW // 2), (3 * QCW + QCW // 2, QCW // 2)]


def _build_nc():
    phases = int(os.environ.get("KERNEL_PHASES", "4"))
    nc = bacc.Bacc("TRN2", target_bir_lowering=False, debug=False, num_devices=8)

    # x tile-major: row m*128+p (p = hid%128 within tile m), col c*128+t, so
    # one token tile is 128 partitions x 4KB contiguous — big DMA descriptors
    xT = nc.dram_tensor("xT", [NT * P, NK * P], BF16, kind="ExternalInput").ap()
    wqT = nc.dram_tensor("wqT", [HID, FQ], BF16, kind="ExternalInput").ap()
    wkvT = nc.dram_tensor("wkvT", [HID, 2 * D], BF16, kind="ExternalInput").ap()
    woT = nc.dram_tensor("woT", [FQ, HID], BF16, kind="ExternalInput").ap()
    cos = nc.dram_tensor("cos", [S, HL, D // 2], BF16, kind="ExternalInput").ap()
    sin = nc.dram_tensor("sin", [S, HL, D // 2], BF16, kind="ExternalInput").ap()
    masks = nc.dram_tensor("masks", [2, P, 2 * QCW], BF16, kind="ExternalInput").ap()
    out = nc.dram_tensor("out", [NQC * P, HID], BF16, kind="ExternalOutput").ap()

    with tile.TileContext(nc) as tc, ExitStack() as ctx:
        dram = ctx.enter_context(tc.tile_pool(name="dram", bufs=1, space="DRAM"))
        const = ctx.enter_context(tc.tile_pool(name="const", bufs=1))

        # attention/out-proj/ReduceScatter run per query piece. The last 512
        # tokens split into two 256-token halves: the first half's RS overlaps
        # the second half's compute, so only a 1MB (not 2MB) RS is exposed.
        # Each ncfw collective op has a ~12us latency floor, so the earlier
        # pieces stay at 512 tokens (fewer, larger ops).
        PIECES = PIECE_LIST
        part_ch = [dram.tile([qw, HID], BF16, name=f"part{i}")
                   for i, (q0, qw) in enumerate(PIECES)]
        # collectives read/write internal DRAM bounce tensors, not IO tensors
        rs_out = [dram.tile([qw // G, HID], BF16, name=f"rsout{i}")
                  for i, (q0, qw) in enumerate(PIECES)]

        # ---- constants ----------------------------------------------------
        # DMA issue order and queue choice matter at startup: wq/wkv gate the
        # first projection matmuls, so they go first on the sync queue; the
        # rest is needed later and rides the vector/scalar queues.
        wpool = ctx.enter_context(tc.tile_pool(name="wts", bufs=1))
        wq_sb = wpool.tile([P, NK, FQ], BF16, name="wq_sb")
        for c in range(NK):
            nc.sync.dma_start(
                out=wq_sb[:, c, :], in_=wqT[c * P:(c + 1) * P, :])
        wkv_sb = wpool.tile([P, NK, 2 * D], BF16, name="wkv_sb")
        nc.sync.dma_start(
            out=wkv_sb[:], in_=wkvT.rearrange("(c p) n -> p c n", p=P))

        # whole x resident in SBUF (no ring stalls); per-tile DMAs issue with
        # a 4-tile prefetch distance so the 8MB doesn't pile onto HBM during
        # the startup weight loads, yet stays ~20us ahead of consumption
        XPREF = 4
        xfull = wpool.tile([P, NT, NK, P], BF16, name="xfull")

        def load_x(m):
            nc.gpsimd.dma_start(
                out=xfull[:, m],
                in_=xT[m * P:(m + 1) * P, :].rearrange("p (c t) -> p c t", c=NK))

        for m in range(XPREF):
            load_x(m)



        ident = const.tile([P, P], BF16, name="ident")
        make_identity(nc, ident)
        epsb = const.tile([P, 1], F32, name="epsb")
        nc.vector.memset(epsb[:], EPS)

        cos_sb = const.tile([P, NT, HL, D // 2], BF16, name="cos_sb")
        nc.scalar.dma_start(
            out=cos_sb[:], in_=cos.rearrange("(m p) h d -> p m h d", p=P))
        sin_sb = const.tile([P, NT, HL, D // 2], BF16, name="sin_sb")
        nc.scalar.dma_start(
            out=sin_sb[:], in_=sin.rearrange("(m p) h d -> p m h d", p=P))
        mask_sb = const.tile([P, 2, 2 * QCW], BF16, name="mask_sb")
        nc.scalar.dma_start(out=mask_sb[:], in_=masks.rearrange("j p f -> p j f"))

        # wo on the scalar queue after cos/sin/mask: each HW DMA queue tops
        # out near ~100GB/s, so x must keep the gpsimd queue to itself
        wo_sb = const.tile([P, HL, HID], BF16, name="wo_sb")
        nc.scalar.dma_start(
            out=wo_sb[:], in_=woT.rearrange("(h p) n -> p h n", p=P))

        qTall = const.tile([P, HL, S], BF16, name="qTall")
        kT = const.tile([P, S], BF16, name="kT")
        vext = [const.tile([P, 129], BF16, name=f"vext{m}") for m in range(NT)]
        for m in range(NT):
            nc.vector.memset(vext[m][:, D:D + 1], 1.0)
        wkp = ctx.enter_context(tc.tile_pool(name="pwork", bufs=2))
        pq = ctx.enter_context(tc.tile_pool(name="pq", bufs=1, space="PSUM"))
        tps = ctx.enter_context(tc.tile_pool(name="tps", bufs=1, space="PSUM"))
        stp = ctx.enter_context(tc.tile_pool(name="stp", bufs=2, space="PSUM"))
        opp = ctx.enter_context(tc.tile_pool(name="opp", bufs=2, space="PSUM"))
        epool = ctx.enter_context(tc.tile_pool(name="epool", bufs=3))
        asb = ctx.enter_context(tc.tile_pool(name="asb", bufs=4))
        rpool = ctx.enter_context(tc.tile_pool(name="rpool", bufs=4))
        att = ctx.enter_context(tc.tile_pool(name="att", bufs=2 * HL))
        osb = ctx.enter_context(tc.tile_pool(name="osb", bufs=2))

        def proj_tile(m):
            # hid-major slice of x for this token tile, already resident:
            # [128 hid, NK chunks, 128 tokens]
            if m + XPREF < NT:
                load_x(m + XPREF)
            xt = xfull[:, m]

            q_ps = pq.tile([P, FQ], F32, tag="q", name=f"q_ps{m}")
            kv_ps = pq.tile([P, 2 * D], F32, tag="kv", name=f"kv_ps{m}")
            for c in range(NK):
                st_ = (c == 0)
                sp_ = (c == NK - 1)
                nc.tensor.matmul(q_ps[:], xt[:, c, :], wq_sb[:, c, :], start=st_, stop=sp_)
                nc.tensor.matmul(kv_ps[:], xt[:, c, :], wkv_sb[:, c, :], start=st_, stop=sp_)

            # casts PSUM->SBUF bf16 on the scalar engine (keeps DVE free)
            qsb = wkp.tile([P, FQ], BF16, tag="qsb", name=f"qsb{m}")
            nc.scalar.copy(out=qsb[:], in_=q_ps[:])
            ksb = wkp.tile([P, D], BF16, tag="ksb", name=f"ksb{m}")
            nc.scalar.copy(out=ksb[:], in_=kv_ps[:, 0:D])
            nc.scalar.copy(out=vext[m][:, 0:D], in_=kv_ps[:, D:2 * D])

            # sum-of-squares per head, computed pre-rope (rope is a rotation:
            # it preserves per-head norms)
            sq = wkp.tile([P, FQ], BF16, tag="sq", name=f"sq{m}")
            nc.vector.tensor_mul(out=sq[:], in0=qsb[:], in1=qsb[:])
            ss = wkp.tile([P, 8], F32, tag="ss", name=f"ss{m}")
            nc.vector.tensor_reduce(
                out=ss[:, 0:HL], in_=sq.rearrange("p (h d) -> p h d", h=HL),
                axis=AxisX, op=AluOp.add)
            sqk = wkp.tile([P, D], BF16, tag="sqk", name=f"sqk{m}")
            nc.vector.tensor_mul(out=sqk[:], in0=ksb[:], in1=ksb[:])
            nc.vector.tensor_reduce(
                out=ss[:, HL:HL + 1], in_=sqk[:], axis=AxisX, op=AluOp.add)
            rs = wkp.tile([P, 8], F32, tag="rs", name=f"rs{m}")
            nc.scalar.activation(
                out=rs[:, 0:HL + 1], in_=ss[:, 0:HL + 1], func=Act.Sqrt,
                scale=1.0 / D, bias=epsb[:])
            rr = wkp.tile([P, 8], F32, tag="rr", name=f"rr{m}")
            nc.vector.reciprocal(out=rr[:, 0:HL + 1], in_=rs[:, 0:HL + 1])

            # rope on q (4 heads at once, bf16, head-repeated cos/sin)
            cosb = cos_sb[:, m, :, :]
            sinb = sin_sb[:, m, :, :]
            qv = qsb.rearrange("p (h two d) -> p h two d", h=HL, two=2)
            qx1 = qv[:, :, 0, :]
            qx2 = qv[:, :, 1, :]
            qn = wkp.tile([P, FQ], BF16, tag="qn", name=f"qn{m}")
            qnv = qn.rearrange("p (h two d) -> p h two d", h=HL, two=2)
            t1 = wkp.tile([P, HL, D // 2], BF16, tag="t1", name=f"t1_{m}")
            t2 = wkp.tile([P, HL, D // 2], BF16, tag="t2", name=f"t2_{m}")
            nc.vector.tensor_mul(out=t1[:], in0=qx1, in1=cosb)
            nc.vector.tensor_mul(out=t2[:], in0=qx2, in1=sinb)
            nc.vector.tensor_add(out=qnv[:, :, 0, :], in0=t1[:], in1=t2[:])
            nc.vector.tensor_mul(out=t1[:], in0=qx2, in1=cosb)
            nc.vector.tensor_mul(out=t2[:], in0=qx1, in1=sinb)
            nc.vector.tensor_sub(out=qnv[:, :, 1, :], in0=t1[:], in1=t2[:])
            qb = wkp.tile([P, FQ], BF16, tag="qb", name=f"qb{m}")
            rrq = rr[:, 0:HL].unsqueeze(2).broadcast_to([P, HL, D])
            nc.vector.tensor_mul(
                out=qb.rearrange("p (h d) -> p h d", h=HL),
                in0=qn.rearrange("p (h d) -> p h d", h=HL), in1=rrq)

            # rope + normalize on k (DVE: gpsimd stays empty so the blocking
            # collective triggers there never stall compute)
            kv_ = ksb.rearrange("p (two d) -> p two d", two=2)
            cosk = cos_sb[:, m, 0, :]
            sink = sin_sb[:, m, 0, :]
            kn = wkp.tile([P, D], BF16, tag="kn", name=f"kn{m}")
            knv = kn.rearrange("p (two d) -> p two d", two=2)
            u1 = wkp.tile([P, D // 2], BF16, tag="u1", name=f"u1_{m}")
            u2 = wkp.tile([P, D // 2], BF16, tag="u2", name=f"u2_{m}")
            nc.vector.tensor_mul(out=u1[:], in0=kv_[:, 0, :], in1=cosk)
            nc.vector.tensor_mul(out=u2[:], in0=kv_[:, 1, :], in1=sink)
            nc.vector.tensor_add(out=knv[:, 0, :], in0=u1[:], in1=u2[:])
            nc.vector.tensor_mul(out=u1[:], in0=kv_[:, 1, :], in1=cosk)
            nc.vector.tensor_mul(out=u2[:], in0=kv_[:, 0, :], in1=sink)
            nc.vector.tensor_sub(out=knv[:, 1, :], in0=u1[:], in1=u2[:])
            kb = wkp.tile([P, D], BF16, tag="kb", name=f"kb{m}")
            rrk = rr[:, HL:HL + 1].broadcast_to([P, D])
            nc.vector.tensor_mul(out=kb[:], in0=kn[:], in1=rrk)

            # PE transposes into qTall / kT
            qtp = tps.tile([P, HL * P], BF16, tag="tp", name=f"qtp{m}")
            for h in range(HL):
                nc.tensor.transpose(
                    qtp[:, h * P:(h + 1) * P], qb[:, h * D:(h + 1) * D], ident[:])
            nc.scalar.copy(
                out=qTall[:, :, m * P:(m + 1) * P],
                in_=qtp.rearrange("p (h w) -> p h w", h=HL))
            tpk = tps.tile([P, P], BF16, tag="tp", name=f"tpk{m}")
            nc.tensor.transpose(tpk[:], kb[:], ident[:])
            nc.scalar.copy(out=kT[:, m * P:(m + 1) * P], in_=tpk[:])

        att_tiles = {}

        def attn_piece(pi, q0, qw):
            kb0 = q0 // P          # first diagonal key tile
            nkb = (q0 + qw) // P
            nqt = qw // P          # query sub-tiles in this piece
            for h in range(HL):
                # 136-stride keeps the second accumulation region 16B-aligned
                osum = opp.tile([P, 2, 136], F32, tag="O", name=f"O{pi}_{h}_a")
                osum2 = opp.tile([P, 2, 136], F32, tag="O", name=f"O{pi}_{h}_b")
                otile = (osum, osum, osum2, osum2)
                for kb in range(nkb):
                    # diagonal key tile kb (offset j>=0 into the piece): only
                    # queries f >= j*128 can attend to it
                    j = kb - kb0
                    f0 = max(j, 0) * P
                    st = stp.tile([P, QCW], F32, tag="st", name=f"st{pi}_{h}_{kb}")
                    nc.tensor.matmul(
                        st[:, f0:qw], kT[:, kb * P:(kb + 1) * P],
                        qTall[:, h, q0 + f0:q0 + qw],
                        start=True, stop=True)
                    ex = epool.tile([P, QCW], BF16, tag="ex",
                                    name=f"ex{pi}_{h}_{kb}")
                    nc.scalar.activation(
                        out=ex[:, f0:qw], in_=st[:, f0:qw], func=Act.Exp,
                        scale=SCALE)
                    if j >= 0:
                        nc.vector.tensor_mul(
                            out=ex[:, f0:qw], in0=ex[:, f0:qw],
                            in1=mask_sb[:, j // 2,
                                        (j % 2) * QCW + f0:(j % 2) * QCW + qw])
                    # visit order 0,2,1,3 alternates PSUM banks between
                    # consecutive accumulating matmuls — same-bank
                    # back-to-back accumulation serializes (~380ns/MM)
                    order = (0, 2, 1, 3) if nqt == 4 else range(nqt)
                    for s in order:
                        if s < j:
                            continue  # query sub-tile fully before key tile
                        # start=True clears the whole PSUM bank's accumulation
                        # state: only the first chain on each bank (s even)
                        # may open the group; the sibling chain's first write
                        # lands in overwrite mode on the freshly cleared bank.
                        # Chain s's last contribution is its diagonal tile.
                        nc.tensor.matmul(
                            otile[s][:, s % 2, 0:129],
                            ex[:, s * P:(s + 1) * P],
                            vext[kb][:],
                            start=(kb == 0 and s % 2 == 0),
                            stop=(kb == kb0 + s))
                # normalize (per-query 1/denom), transpose to feature-major
                nA = min(nqt, 2)
                rcA = rpool.tile([P, 2], F32, tag="rcA", name=f"rcA{pi}_{h}")
                nc.vector.reciprocal(out=rcA[:, 0:nA], in_=osum[:, 0:nA, D])
                obA = asb.tile([P, 2, D], BF16, tag="obA", name=f"obA{pi}_{h}")
                nc.vector.tensor_mul(
                    out=obA[:, 0:nA], in0=osum[:, 0:nA, 0:D],
                    in1=rcA[:, 0:nA].unsqueeze(2).broadcast_to([P, nA, D]))
                obs = [obA[:, 0, :], obA[:, 1, :]]
                if nqt > 2:
                    rcB = rpool.tile([P, 2], F32, tag="rcB", name=f"rcB{pi}_{h}")
                    nc.vector.reciprocal(out=rcB[:], in_=osum2[:, :, D])
                    obB = asb.tile([P, 2, D], BF16, tag="obB",
                                   name=f"obB{pi}_{h}")
                    nc.vector.tensor_mul(
                        out=obB[:], in0=osum2[:, :, 0:D],
                        in1=rcB.unsqueeze(2).broadcast_to([P, 2, D]))
                    obs += [obB[:, 0, :], obB[:, 1, :]]
                to4 = tps.tile([P, QCW], BF16, tag="tp", name=f"to{pi}_{h}")
                for s in range(nqt):
                    nc.tensor.transpose(to4[:, s * P:(s + 1) * P], obs[s], ident[:])
                att_h = att.tile([P, QCW], BF16, tag="attn", name=f"attn{pi}_{h}")
                nc.vector.tensor_copy(out=att_h[:, 0:qw], in_=to4[:, 0:qw])
                att_tiles[(pi, h)] = att_h

        grp = [[0, 1, 2, 3], [4, 5, 6, 7]]

        def outproj_piece(pi, q0, qw, ro, phases):
            # partial out-proj: contract only the local 512 features; the
            # ReduceScatter across the 4 same-batch cores supplies the rest
            # (and scatters qw/4 tokens back to each core).
            stores = []
            for m in range(qw // P):
                for obp in range(NOB // 2):
                    # two column banks accumulate in lockstep across the two
                    # PSUM ring slots: consecutive matmuls alternate banks
                    # (same-bank accumulation serializes) and share each
                    # LDWEIGHTS (the stationary depends on (h, m) only).
                    # The pool is shared with the scores ring: attention and
                    # out-proj are phase-disjoint, so the ring pipelines.
                    oa, ob_ = 2 * obp, 2 * obp + 1
                    poA = stp.tile([P, QCW], F32, tag="st", name=f"po{pi}_{m}_{oa}")
                    poB = stp.tile([P, QCW], F32, tag="st", name=f"po{pi}_{m}_{ob_}")
                    for h in range(HL):
                        aslice = att_tiles[(pi, h)][:, m * P:(m + 1) * P]
                        nc.tensor.matmul(
                            poA[:], aslice, wo_sb[:, h, oa * QCW:(oa + 1) * QCW],
                            start=(h == 0), stop=(h == HL - 1))
                        nc.tensor.matmul(
                            poB[:], aslice, wo_sb[:, h, ob_ * QCW:(ob_ + 1) * QCW],
                            start=(h == 0), stop=(h == HL - 1))
                    for ob, po in ((oa, poA), (ob_, poB)):
                        ot = osb.tile([P, QCW], BF16, tag="ot",
                                      name=f"ot{pi}_{m}_{ob}")
                        nc.vector.tensor_copy(out=ot[:], in_=po[:])
                        stores.append(nc.sync.dma_start(
                            out=part_ch[pi][m * P:(m + 1) * P,
                                            ob * QCW:(ob + 1) * QCW],
                            in_=ot[:]))
            if phases >= 4:
                cc = nc.gpsimd.collective_compute(
                    "ReduceScatter", AluOp.add, replica_groups=grp,
                    ins=[part_ch[pi].opt()], outs=[rs_out[pi].opt()])
                for st_ in stores:
                    tile.add_dep_helper(
                        cc.ins, st_.ins, sync=True,
                        reason="ReduceScatter reads partial stores")
                # out copy rides the gpsimd stream: it must wait for the
                # collective anyway, and gpsimd (which blocks on cc) carries
                # no compute — on sync it head-of-line blocked the next
                # piece's partial stores for the whole collective duration
                dinst = nc.gpsimd.dma_start(
                    out=out[ro:ro + qw // G, :], in_=rs_out[pi][:])
                tile.add_dep_helper(
                    dinst.ins, cc.ins, sync=True,
                    reason="out copy reads ReduceScatter output")

        done = 0
        ro = 0
        for pi, (q0, qw) in enumerate(PIECES):
            for m in range(done, (q0 + qw) // P):
                proj_tile(m)
            done = (q0 + qw) // P
            if phases >= 2:
                attn_piece(pi, q0, qw)
                if phases >= 3:
                    outproj_piece(pi, q0, qw, ro, phases)
            ro += qw // G

    nc.compile()
    return nc


_NC_CACHE = {}


def _get_nc():
    if "nc" not in _NC_CACHE:
        _NC_CACHE["nc"] = _build_nc()
    return _NC_CACHE["nc"]


def _make_masks():
    # masks[jj2][p, jj*QCW + f] = 1 iff query f >= key offset (2*jj2+jj)*128+p
    out = np.zeros((2, P, 2 * QCW), dtype=np.float32)
    p = np.arange(P)[:, None]
    f = np.arange(QCW)[None, :]
    for jj2 in range(2):
        for jj in range(2):
            j = 2 * jj2 + jj
            out[jj2][:, jj * QCW:(jj + 1) * QCW] = (f >= j * P + p)
    return out.astype(ml_dtypes.bfloat16)


def kernel(**inputs):
    x = np.asarray(inputs["x"], np.float32)
    cos = np.asarray(inputs["cos"], np.float32).reshape(S, D // 2)
    sin = np.asarray(inputs["sin"], np.float32).reshape(S, D // 2)
    Wq = np.asarray(inputs["Wq"], np.float32)
    Wk = np.asarray(inputs["Wk"], np.float32)
    Wv = np.asarray(inputs["Wv"], np.float32)
    Wo = np.asarray(inputs["Wo"], np.float32)

    masks = _make_masks()
    bf = ml_dtypes.bfloat16

    # tile-major x: arr[m, p, c, t] = x[b][m*128+t, c*128+p] so a token tile's
    # DMA reads 4KB contiguous per partition
    xTb = [
        np.ascontiguousarray(
            x[b].reshape(NT, P, NK, P).transpose(0, 3, 2, 1)
        ).reshape(NT * P, NK * P).astype(bf)
        for b in range(B)
    ]
    cosb = np.ascontiguousarray(
        np.repeat(cos[:, None, :], HL, axis=1)).astype(bf)
    sinb = np.ascontiguousarray(
        np.repeat(sin[:, None, :], HL, axis=1)).astype(bf)

    in_maps = []
    for c in range(8):
        b, g = divmod(c, G)
        in_maps.append({
            "xT": xTb[b],
            "wqT": np.ascontiguousarray(Wq[g * FQ:(g + 1) * FQ, :].T).astype(bf),
            "wkvT": np.ascontiguousarray(np.concatenate([
                Wk[g * D:(g + 1) * D, :].T,
                Wv[g * D:(g + 1) * D, :].T], axis=1)).astype(bf),
            "woT": np.ascontiguousarray(Wo[:, g * FQ:(g + 1) * FQ].T).astype(bf),
            "cos": cosb,
            "sin": sinb,
            "masks": masks,
        })

    nc = _get_nc()
    trace = bool(int(os.environ.get("KERNEL_TRACE", "0")))
    tmpdir = os.environ.get("KERNEL_TMPDIR") or None
    res = run_bass_kernel_spmd(
        nc, in_maps, core_ids=list(range(8)), trace=trace, tmpdir=tmpdir)
    kernel.exec_time_ns = res.exec_time_ns
    kernel.last_result = res

    out = np.empty((B, S, HID), np.float32)
    for c in range(8):
        b, r = divmod(c, G)
        o = np.asarray(res.results[c]["out"]).astype(np.float32)
        ro = 0
        for q0, qw in PIECE_LIST:
            n = qw // G
            t0 = q0 + r * n
            out[b, t0:t0 + n, :] = o[ro:ro + n]
            ro += n
    return out
